# revision 1
# baseline (speedup 1.0000x reference)
"""Multi-head attention Trainium2 kernel, 8-way sharded.

Problem: x[4,2048,1024] -> qkv proj (w_qkv [3072,1024]) -> 16-head attention
with key-padding mask -> tail proj (w_tail [1024,1024]) + b_tail.

Sharding: 8 shards = 4 batches x 2 head-groups (8 heads each). Each core
computes, for its (batch b, head-group hg):
  - q/k/v projections of x[b] for its 8 heads
  - full [2048 x 2048] masked attention per head
  - partial tail matmul y_part = attn_cat @ w_tail[:, cat_slice].T
Host unshards: out[b] = y_part[2b] + y_part[2b+1] + b_tail.  No collectives.

Layouts (per core, all weights pre-transposed on host):
  xT      [1024, 2048]  x[b].T
  wqkT    [1024, 1024]  q|k rows (128/head) of w_qkv shard, transposed
  wvT     [1024,  512]  v rows (64/head) of w_qkv shard, transposed
  wtailT  [ 512, 1024]  w_tail[:, hg*512:(hg+1)*512].T
  mask    [2048] int32
Kernel computes qT/kT per head via W @ xT, V directly as x @ Wv^T (token-major),
streams S^T = K Q^T per 128-key block, exp via ACT with the mask folded in as a
per-partition bias, accumulates attn^T (+ denominator via a ones column on V)
on PE, normalizes via PE transposes + per-token reciprocal, and finishes with
the tail matmul from the stacked normalized attn^T.
"""

import time as _time

import numpy as np
from contextlib import ExitStack

import concourse.bass as bass
import concourse.mybir as mybir
import concourse.tile as tile
from concourse.bass_utils import run_bass_kernel_spmd

# ---------------------------------------------------------------------------
# walrus in this env accepts at most 2 sync waits per instruction; Tile's
# scheduler emits up to 10. Post-pass: peel excess waits onto same-engine
# NoOps inserted immediately before the offending instruction (same engine
# stream position => identical synchronization semantics).
MAX_WAITS = 1


def split_excess_waits(nc):
    for fn in nc.m.functions:
        for bb in fn.blocks:
            insts = list(bb.instructions)
            out = []
            changed = False
            for inst in insts:
                si = inst.sync_info
                waits = list(si.on_wait) if si is not None else []
                if len(waits) > MAX_WAITS:
                    extra = waits[:-MAX_WAITS]
                    for ci in range(0, len(extra), MAX_WAITS):
                        chunk = extra[ci:ci + MAX_WAITS]
                        nop = mybir.InstNoOp(
                            name=f"{inst.name}-ws{ci}", ins=[], outs=[])
                        nop.engine = inst.engine
                        nop.sync_info = mybir.SyncInfo(
                            on_wait=chunk, on_update=[])
                        out.append(nop)
                    inst.sync_info = mybir.SyncInfo(
                        on_wait=waits[-MAX_WAITS:],
                        on_update=list(si.on_update))
                    changed = True
                out.append(inst)
            if changed:
                bb.instructions = out
# ---------------------------------------------------------------------------

D_MODEL = 1024
N_HEAD = 16
D_HEAD = 64
BN, T = 4, 2048
HPC = 8                      # heads per core
CAT = HPC * D_HEAD           # 512 per-core tail contraction
NKB = T // 128               # 16 key blocks
NTB = T // 128               # 16 token blocks
QH = T // 2                  # 1024, q processed in two halves
KC = D_MODEL // 128          # 8 contraction chunks
F32 = mybir.dt.float32
I32 = mybir.dt.int32

# matmul compute dtype: float32 (exact, 4 cyc/row) or float32r (1 cyc/row)
import os as _os
MM_DT = (mybir.dt.float32 if _os.environ.get("MHA_MM_DT", "f32r") == "f32"
         else mybir.dt.float32r)


MDT = MM_DT  # dtype for all matmul-operand tiles (producers round to it)


def _mm(ap):
    return ap


def build_nc(split_waits=True):
    nc = bass.Bass()
    xT = nc.declare_dram_parameter("xT", [D_MODEL, T], MDT, isOutput=False)
    wqkT = nc.declare_dram_parameter("wqkT", [D_MODEL, HPC * 128], MDT, isOutput=False)
    wvT = nc.declare_dram_parameter("wvT", [D_MODEL, CAT], MDT, isOutput=False)
    wtailT = nc.declare_dram_parameter("wtailT", [CAT, D_MODEL], MDT, isOutput=False)
    mask = nc.declare_dram_parameter("mask", [T], I32, isOutput=False)
    ident = nc.declare_dram_parameter("ident", [128, 128], F32, isOutput=False)
    ones8 = nc.declare_dram_parameter("ones8", [128, HPC], MDT, isOutput=False)
    y = nc.declare_dram_parameter("y", [T, D_MODEL], F32, isOutput=True)

    with ExitStack() as ctx:
        tc = ctx.enter_context(tile.TileContext(nc))

        # ---- long-lived pools (entered first so short-lived ones stack on top)
        const = ctx.enter_context(tc.tile_pool(name="const", bufs=1))
        qk_pool = ctx.enter_context(tc.tile_pool(name="qk", bufs=1))
        vaug_pool = ctx.enter_context(tc.tile_pool(name="vaug", bufs=1))

        identity = const.tile([128, 128], F32)
        nc.sync.dma_start(out=identity, in_=ident[:, :])

        # mask -> per-key-block additive bias: (m-1)*8e9  (0 keep, -8e9 drop)
        mask_i = const.tile([128, NKB], I32)
        nc.sync.dma_start(out=mask_i, in_=mask.rearrange("(j p) -> p j", p=128))
        maskb = const.tile([128, NKB], F32)
        nc.vector.tensor_copy(out=maskb, in_=mask_i)
        nc.vector.tensor_scalar(
            out=maskb, in0=maskb, scalar1=-1.0, scalar2=8e9,
            op0=mybir.AluOpType.add, op1=mybir.AluOpType.mult,
        )

        # persistent intermeds
        # q/k of 2 heads per tile: rows (h%2)*64..+64
        qts = [qk_pool.tile([128, T], MDT, tag=f"qt{j}", name=f"qt{j}") for j in range(HPC // 2)]
        kts = [qk_pool.tile([128, T], MDT, tag=f"kt{j}", name=f"kt{j}") for j in range(HPC // 2)]
        # V augmented with ones column: [tok-block][128, head, 65]
        vaugs = [vaug_pool.tile([128, HPC, D_HEAD + 1], MDT, tag=f"va{t}", name=f"va{t}")
                 for t in range(NTB)]
        # ---- phase 1: projections (xT resident, freed afterwards)
        with tc.tile_pool(name="xp", bufs=1) as xp_pool:
            xts = [xp_pool.tile([128, T], MDT, tag=f"x{kc}", name=f"x{kc}") for kc in range(KC)]
            for kc in range(KC):
                nc.sync.dma_start(out=xts[kc][:, 0:QH],
                                  in_=xT[kc * 128:(kc + 1) * 128, 0:QH])
            for kc in range(KC):
                nc.sync.dma_start(out=xts[kc][:, QH:T],
                                  in_=xT[kc * 128:(kc + 1) * 128, QH:T])

            # V projection: V[tok, cat] = x @ Wv^T ; ones column appended.
            # kc-outer with 8 live PSUM banks per tb-group so wv streams.
            with tc.tile_pool(name="wv", bufs=2) as wv_pool, \
                 tc.tile_pool(name="vps", bufs=1, space="PSUM") as vps:
                for grp in range(2):
                    vp8 = [vps.tile([128, CAT], F32, tag=f"vp{i}", name=f"vp{i}")
                           for i in range(8)]
                    for kc in range(KC):
                        wv = wv_pool.tile([128, CAT], MDT, tag="wv", name="wv")
                        nc.sync.dma_start(
                            out=wv, in_=wvT[kc * 128:(kc + 1) * 128, :])
                        for i in range(8):
                            tb = grp * 8 + i
                            nc.tensor.matmul(
                                vp8[i],
                                _mm(xts[kc][:, tb * 128:(tb + 1) * 128]),
                                _mm(wv),
                                start=(kc == 0), stop=(kc == KC - 1),
                            )
                    for i in range(8):
                        tb = grp * 8 + i
                        va = vaugs[tb]
                        nc.sync.dma_start(
                            out=va[:, :, D_HEAD:D_HEAD + 1], in_=ones8[:, :])
                        nc.vector.tensor_copy(
                            out=va[:, :, 0:D_HEAD],
                            in_=vp8[i].rearrange("p (h d) -> p h d", h=HPC),
                        )

            # q/k projection per head: qkT = Wqk_h @ xT  -> [128 rows, T]
            with tc.tile_pool(name="wqk", bufs=1) as wqk_pool, \
                 tc.tile_pool(name="qkps", bufs=1, space="PSUM") as qkps, \
                 tc.tile_pool(name="dps1", bufs=1, space="PSUM") as dps1:

                def warm_keeper1():
                    dmy1 = dps1.tile([128, 128], F32, tag="dmy1", name="dmy1")
                    nc.tensor.matmul(dmy1, identity, identity, start=True, stop=True)
                wqs = [wqk_pool.tile([128, KC, 128], MDT, tag=f"wqk{h}",
                                     name=f"wq{h}") for h in range(HPC)]
                for h in range(HPC):
                    nc.sync.dma_start(
                        out=wqs[h],
                        in_=wqkT.rearrange("(kc p) c -> p kc c", p=128)[
                            :, :, h * 128:(h + 1) * 128],
                    )
                for h in range(HPC):
                    wq = wqs[h]
                    j, r0 = h // 2, (h % 2) * 64
                    for nh in range(2):
                        qkp = qkps.tile([128, T // 2], F32, tag="qkp",
                                        name="qkp", bufs=2)
                        warm_keeper1()
                        for n in range(2):
                            for kc in range(KC):
                                nc.tensor.matmul(
                                    qkp[:, n * 512:(n + 1) * 512],
                                    _mm(wq[:, kc, :]),
                                    _mm(xts[kc][:, nh * 1024 + n * 512:
                                                nh * 1024 + (n + 1) * 512]),
                                    start=(kc == 0), stop=(kc == KC - 1),
                                )
                        q0 = nh * 1024
                        nc.vector.tensor_copy(
                            out=qts[j][r0:r0 + 64, q0:q0 + 1024],
                            in_=qkp[0:64, :])
                        nc.vector.tensor_copy(
                            out=kts[j][r0:r0 + 64, q0:q0 + 1024],
                            in_=qkp[64:128, :])

        # ---- phase 2: attention per head, q in two halves
        num_pool = ctx.enter_context(tc.tile_pool(name="num", bufs=1))
        # stacked normalized attn^T: 2 heads per tile (cat rows)
        nums = [num_pool.tile([128, T], MDT, tag=f"nm{j}", name=f"nm{j}")
                for j in range(CAT // 128)]
        with tc.tile_pool(name="p_sb", bufs=5) as p_pool, \
             tc.tile_pool(name="av_sb", bufs=3) as avsb_pool, \
             tc.tile_pool(name="r_sb", bufs=4) as r_pool, \
             tc.tile_pool(name="at_sb", bufs=2) as at_pool, \
             tc.tile_pool(name="stps", bufs=2, space="PSUM") as stps, \
             tc.tile_pool(name="avps", bufs=1, space="PSUM") as avps, \
             tc.tile_pool(name="tps", bufs=1, space="PSUM") as tps, \
             tc.tile_pool(name="dps", bufs=1, space="PSUM") as dps:

            def warm_keeper():
                dmy = dps.tile([128, 128], F32, tag="dmy", name="dmy")
                nc.tensor.matmul(dmy, identity, identity, start=True, stop=True)
            # Software-pipelined emission: within a unit (head, q-half) the
            # PE stream is ST(0),ST(1),...,ST(kb),AV(kb-2),... so the PE
            # always has a queued matmul while ACT computes exp; the
            # normalize (transpose) work of the previous unit is emitted
            # early in the next unit to fill the exp-latency window.
            LAG = 4

            def normalize_unit(av_sb, ap_tile, r0):
                for tb in range(QH // 128):
                    t1 = tps.tile([128, 128], F32, tag="tp", name="t1")
                    nc.tensor.transpose(
                        t1[:, 0:D_HEAD + 1],
                        av_sb[:, tb * 128:(tb + 1) * 128],
                        identity[0:D_HEAD + 1, 0:D_HEAD + 1],
                    )
                    r_sb = r_pool.tile([128, 1], F32, tag="r", name="r_sb")
                    nc.vector.reciprocal(out=r_sb, in_=t1[:, D_HEAD:D_HEAD + 1])
                    nc.vector.tensor_scalar_mul(
                        ap_tile[:, tb, r0:r0 + 64], t1[:, 0:D_HEAD], r_sb)

            def flush_pair(aps, j):
                for half in range(2):
                    q0 = half * QH
                    for tb in range(QH // 128):
                        t2 = tps.tile([128, 128], F32, tag="tp", name="t2")
                        nc.tensor.transpose(t2, aps[half][:, tb, :], identity)
                        nc.vector.tensor_copy(
                            out=nums[j][:, q0 + tb * 128:q0 + (tb + 1) * 128],
                            in_=t2,
                        )

            pending_norm = None   # (av_sb, ap_tile, r0)
            pending_pair = None   # (aps, j)
            cur_aps = None
            for pair in range(HPC // 2):
                # token-major normalized attn for the head pair, per q-half:
                # [tok-part, tok-blk, cat(2 heads x 64)]
                cur_aps = [at_pool.tile([128, QH // 128, 128], F32,
                                        tag=f"ap{hf}", name=f"ap{hf}")
                           for hf in range(2)]
                for sub in range(2):
                    h = 2 * pair + sub
                    r0 = sub * 64
                    qt = qts[pair][r0:r0 + 64, :]
                    kt = kts[pair][r0:r0 + 64, :]
                    for half in range(2):
                        q0 = half * QH
                        avp = avps.tile([D_HEAD + 1, QH], F32, tag="avp",
                                        name="avp")
                        p_tiles = {}

                        def emit_st_exp(kb):
                            stp = stps.tile([128, QH], F32, tag="stp",
                                            name="stp")
                            for n in range(QH // 512):
                                nc.tensor.matmul(
                                    stp[:, n * 512:(n + 1) * 512],
                                    _mm(kt[:, kb * 128:(kb + 1) * 128]),
                                    _mm(qt[:, q0 + n * 512:q0 + (n + 1) * 512]),
                                    start=True, stop=True,
                                )
                            p_sb = p_pool.tile([128, QH], MDT, tag="p",
                                               name="p_sb")
                            nc.scalar.activation(
                                out=p_sb, in_=stp,
                                func=mybir.ActivationFunctionType.Exp,
                                bias=maskb[:, kb:kb + 1], scale=0.125,
                            )
                            p_tiles[kb] = p_sb

                        def emit_av(kb):
                            p_sb = p_tiles.pop(kb)
                            for n in range(QH // 512):
                                nc.tensor.matmul(
                                    avp[:, n * 512:(n + 1) * 512],
                                    _mm(vaugs[kb][:, h, :]),
                                    _mm(p_sb[:, n * 512:(n + 1) * 512]),
                                    start=(kb == 0), stop=(kb == NKB - 1),
                                )

                        for kb in range(LAG):
                            if kb % 2 == 0:
                                warm_keeper()
                            emit_st_exp(kb)
                        # fill the exp latency with deferred PE work
                        if pending_norm is not None:
                            normalize_unit(*pending_norm)
                            pending_norm = None
                        if pending_pair is not None:
                            flush_pair(*pending_pair)
                            pending_pair = None
                        for kb in range(LAG, NKB):
                            if kb % 2 == 0:
                                warm_keeper()
                            emit_st_exp(kb)
                            emit_av(kb - LAG)
                        for kb in range(NKB - LAG, NKB):
                            emit_av(kb)
                        av_sb = avsb_pool.tile([D_HEAD + 1, QH], F32,
                                               tag="avsb", name="av_sb")
                        nc.vector.tensor_copy(out=av_sb, in_=avp)
                        pending_norm = (av_sb, cur_aps[half], r0)
                pending_pair = (cur_aps, pair)
            # drain the pipeline
            if pending_norm is not None:
                normalize_unit(*pending_norm)
            if pending_pair is not None:
                flush_pair(*pending_pair)

        # ---- phase 3: tail matmul  y[tok, out] = attn_cat @ wtailT
        with tc.tile_pool(name="wt", bufs=1) as wt_pool, \
             tc.tile_pool(name="y_sb", bufs=3) as y_pool, \
             tc.tile_pool(name="yps", bufs=2, space="PSUM") as yps, \
             tc.tile_pool(name="dps3", bufs=1, space="PSUM") as dps3:

            def warm_keeper3():
                dmy3 = dps3.tile([128, 128], F32, tag="dmy3", name="dmy3")
                nc.tensor.matmul(dmy3, identity, identity, start=True, stop=True)
            wts = [wt_pool.tile([128, D_MODEL], MDT, tag=f"wt{c}", name=f"wt{c}")
                   for c in range(CAT // 128)]
            for c in range(CAT // 128):
                nc.sync.dma_start(out=wts[c], in_=wtailT[c * 128:(c + 1) * 128, :])
            for tb in range(NTB):
                warm_keeper3()
                yp = yps.tile([128, D_MODEL], F32, tag="yp")
                for n in range(D_MODEL // 512):
                    for c in range(CAT // 128):
                        nc.tensor.matmul(
                            yp[:, n * 512:(n + 1) * 512],
                            _mm(nums[c][:, tb * 128:(tb + 1) * 128]),
                            _mm(wts[c][:, n * 512:(n + 1) * 512]),
                            start=(c == 0), stop=(c == CAT // 128 - 1),
                        )
                y_sb = y_pool.tile([128, D_MODEL], F32, tag="ys")
                nc.vector.tensor_copy(out=y_sb, in_=yp)
                nc.sync.dma_start(out=y[tb * 128:(tb + 1) * 128, :], in_=y_sb)

    if split_waits:
        split_excess_waits(nc)
    return nc


_NC_CACHE = None


def _get_nc():
    global _NC_CACHE
    if _NC_CACHE is None:
        _NC_CACHE = build_nc()
    return _NC_CACHE


def make_in_maps(x, mask, w_qkv, w_tail):
    """Shard full inputs into 8 per-core input maps."""
    x = np.asarray(x, dtype=np.float32)
    mask = np.asarray(mask, dtype=np.int32)
    w_qkv = np.asarray(w_qkv, dtype=np.float32)
    w_tail = np.asarray(w_tail, dtype=np.float32)

    w3 = w_qkv.reshape(N_HEAD, 3, D_HEAD, D_MODEL)  # [head, qkv, d, dmodel]
    in_maps = []
    for c in range(8):
        b, hg = c // 2, c % 2
        heads = range(hg * HPC, (hg + 1) * HPC)
        wqk = np.concatenate(
            [w3[h, 0:2].reshape(128, D_MODEL) for h in heads], axis=0
        )  # [1024, 1024] rows = (head-local, q|k, d)
        wv = np.concatenate([w3[h, 2] for h in heads], axis=0)  # [512, 1024]
        wt = w_tail[:, hg * CAT:(hg + 1) * CAT]  # [1024, 512]
        in_maps.append({
            "ident": np.eye(128, dtype=np.float32),
            "ones8": np.ones((128, HPC), dtype=np.float32),
            "xT": np.ascontiguousarray(x[b].T),
            "wqkT": np.ascontiguousarray(wqk.T),
            "wvT": np.ascontiguousarray(wv.T),
            "wtailT": np.ascontiguousarray(wt.T),
            "mask": mask[b],
        })
    return in_maps


def kernel(x, mask, w_qkv, w_tail, b_tail):
    nc = _get_nc()
    in_maps = make_in_maps(x, mask, w_qkv, w_tail)
    last_err = None
    for _attempt in range(3):
        try:
            res = run_bass_kernel_spmd(nc, in_maps, list(range(8))).results
            break
        except Exception as e:  # transient device/runtime errors: retry
            last_err = e
            _time.sleep(3.0)
    else:
        raise last_err
    out = np.empty((BN, T, D_MODEL), dtype=np.float32)
    b_tail = np.asarray(b_tail, dtype=np.float32)
    for b in range(BN):
        out[b] = res[2 * b]["y"] + res[2 * b + 1]["y"] + b_tail
    return out



# revision 3
# speedup vs baseline: 1.4394x; 1.4394x over previous
"""Multi-head attention Trainium2 kernel, 8-way sharded. v2.

Problem: x[4,2048,1024] -> qkv proj (w_qkv [3072,1024]) -> 16-head attention
with key-padding mask -> tail proj (w_tail [1024,1024]) + b_tail.

Sharding: 8 shards = 4 batches x 2 head-groups (8 heads each = 4 head PAIRS).
Host unshards: out[b] = y_part[2b] + y_part[2b+1] + b_tail.  No collectives.

Key ideas vs v1 (603 us):
- Host-side key gather: masked keys (~50%) are dropped before the kernel;
  only ceil(max_kept/128) key blocks are computed. Mask correctness comes
  from zeroing V rows and the denominator-ones column for pad keys, so exp
  needs no bias at all.
- bf16 matmul operands everywhere (fp32r matmuls self-load weights serially,
  ~175 ns/matmul; bf16 gets separate LDWEIGHTS that the PE reorder window
  hides under the previous matmul's stream).
- ST row-tiling: the two heads of a pair have K=64 contractions, placed in
  array rows 0-63 / 64-127 via tile_position -> both score matmuls run
  concurrently.
- One exp per (pair, kb, qb): N=1024 covering both heads' [128 keys x 512 q]
  scores in a 2-bank PSUM tile.
- Normalization without PE transposes: denominator row is reciprocal'd,
  broadcast to 128 partitions with a tiny selector matmul, and applied
  during the PSUM->SBUF evacuation of attn^T.
- Single software-pipelined emission stream: projection / tail matmul
  quanta are injected between attention steps (with explicit deadlines)
  so the PE fills the slack under the Act(exp) stream; phases overlap.
"""

import time as _time

import numpy as np
import ml_dtypes
from contextlib import ExitStack

import concourse.bass as bass
import concourse.mybir as mybir
import concourse.tile as tile
from concourse.bass_utils import run_bass_kernel_spmd

# ---------------------------------------------------------------------------
# walrus in this env accepts at most 2 sync waits per instruction; Tile's
# scheduler emits up to 10. Post-pass: peel excess waits onto same-engine
# NoOps inserted immediately before the offending instruction (same engine
# stream position => identical synchronization semantics).
MAX_WAITS = 1


def split_excess_waits(nc):
    for fn in nc.m.functions:
        for bb in fn.blocks:
            insts = list(bb.instructions)
            out = []
            changed = False
            for inst in insts:
                si = inst.sync_info
                waits = list(si.on_wait) if si is not None else []
                if len(waits) > MAX_WAITS:
                    extra = waits[:-MAX_WAITS]
                    for ci in range(0, len(extra), MAX_WAITS):
                        chunk = extra[ci:ci + MAX_WAITS]
                        nop = mybir.InstNoOp(
                            name=f"{inst.name}-ws{ci}", ins=[], outs=[])
                        nop.engine = inst.engine
                        nop.sync_info = mybir.SyncInfo(
                            on_wait=chunk, on_update=[])
                        out.append(nop)
                    inst.sync_info = mybir.SyncInfo(
                        on_wait=waits[-MAX_WAITS:],
                        on_update=list(si.on_update))
                    changed = True
                out.append(inst)
            if changed:
                bb.instructions = out
# ---------------------------------------------------------------------------

D_MODEL = 1024
N_HEAD = 16
D_HEAD = 64
BN, T = 4, 2048
HPC = 8                      # heads per core
NPAIR = HPC // 2             # 4 head pairs per core
CAT = HPC * D_HEAD           # 512 per-core tail contraction
KC = D_MODEL // 128          # 8 contraction chunks
NQB = T // 512               # 4 q blocks of 512
F32 = mybir.dt.float32
F32R = mybir.dt.float32r
BF16 = mybir.dt.bfloat16
BF = ml_dtypes.bfloat16


def build_nc(nkb, split_waits=True):
    """nkb: number of 128-key blocks of gathered (kept+pad) keys."""
    KT = nkb * 128
    NKC = (KT + 511) // 512      # kproj 512-col chunks
    nc = bass.Bass()
    xqT = nc.declare_dram_parameter("xqT", [D_MODEL, T], BF16, isOutput=False)
    xkT = nc.declare_dram_parameter("xkT", [D_MODEL, KT], BF16, isOutput=False)
    wqT = nc.declare_dram_parameter("wqT", [D_MODEL, NPAIR * 128], BF16, isOutput=False)
    wkT = nc.declare_dram_parameter("wkT", [D_MODEL, NPAIR * 128], BF16, isOutput=False)
    wvT = nc.declare_dram_parameter("wvT", [D_MODEL, CAT], BF16, isOutput=False)
    wtailT = nc.declare_dram_parameter("wtailT", [CAT, D_MODEL], BF16, isOutput=False)
    keep = nc.declare_dram_parameter("keep", [KT], F32, isOutput=False)
    ones8 = nc.declare_dram_parameter("ones8", [128, HPC], BF16, isOutput=False)
    sel = nc.declare_dram_parameter("sel", [33, 128], F32R, isOutput=False)
    y = nc.declare_dram_parameter("y", [T, D_MODEL], F32, isOutput=True)

    with ExitStack() as ctx:
        tc = ctx.enter_context(tile.TileContext(nc))

        const = ctx.enter_context(tc.tile_pool(name="const", bufs=1))
        xq_pool = ctx.enter_context(tc.tile_pool(name="xq", bufs=1))
        xk_pool = ctx.enter_context(tc.tile_pool(name="xk", bufs=1))
        qk_pool = ctx.enter_context(tc.tile_pool(name="qk", bufs=1))
        va_pool = ctx.enter_context(tc.tile_pool(name="va", bufs=1))
        num_pool = ctx.enter_context(tc.tile_pool(name="num", bufs=1))
        w_pool = ctx.enter_context(tc.tile_pool(name="w", bufs=1))
        p_pool = ctx.enter_context(tc.tile_pool(name="p", bufs=1))
        stag_pool = ctx.enter_context(tc.tile_pool(name="stag", bufs=1))
        rab_pool = ctx.enter_context(tc.tile_pool(name="rab", bufs=1))
        ysb_pool = ctx.enter_context(tc.tile_pool(name="ysb", bufs=1))
        stp_pool = ctx.enter_context(tc.tile_pool(name="stp", bufs=1, space="PSUM"))
        avp_pool = ctx.enter_context(tc.tile_pool(name="avp", bufs=1, space="PSUM"))
        aux_pool = ctx.enter_context(tc.tile_pool(name="aux", bufs=1, space="PSUM"))

        # ---- consts
        selt = const.tile([33, 128], F32R, name="sel")
        nc.sync.dma_start(out=selt, in_=sel[:, :])
        keepc = const.tile([128, nkb], F32, name="keep")
        nc.sync.dma_start(out=keepc, in_=keep.rearrange("(j p) -> p j", p=128))
        ones8t = const.tile([128, HPC], BF16, name="ones8")
        nc.sync.dma_start(out=ones8t, in_=ones8[:, :])

        # ---- inputs (order = DMA priority)
        xks = [xk_pool.tile([128, KT], BF16, tag=f"xk{kc}", name=f"xk{kc}")
               for kc in range(KC)]
        for kc in range(KC):
            nc.sync.dma_start(out=xks[kc][:, 0:512],
                              in_=xkT[kc * 128:(kc + 1) * 128, 0:512])
        wvs = [w_pool.tile([128, CAT], BF16, tag=f"wv{kc}", name=f"wv{kc}")
               for kc in range(KC)]
        for kc in range(KC):
            nc.sync.dma_start(out=wvs[kc], in_=wvT[kc * 128:(kc + 1) * 128, :])
        wks = [w_pool.tile([128, KC, 128], BF16, tag=f"wk{j}", name=f"wk{j}")
               for j in range(NPAIR)]
        wqs = [w_pool.tile([128, KC, 128], BF16, tag=f"wq{j}", name=f"wq{j}")
               for j in range(NPAIR)]
        for j in range(NPAIR):
            nc.sync.dma_start(
                out=wks[j],
                in_=wkT.rearrange("(kc p) c -> p kc c", p=128)[
                    :, :, j * 128:(j + 1) * 128])
            nc.sync.dma_start(
                out=wqs[j],
                in_=wqT.rearrange("(kc p) c -> p kc c", p=128)[
                    :, :, j * 128:(j + 1) * 128])
        if KT > 512:
            for kc in range(KC):
                nc.sync.dma_start(out=xks[kc][:, 512:KT],
                                  in_=xkT[kc * 128:(kc + 1) * 128, 512:KT])
        xqs = [xq_pool.tile([128, T], BF16, tag=f"xq{kc}", name=f"xq{kc}")
               for kc in range(KC)]
        for kc in range(KC):
            nc.sync.dma_start(out=xqs[kc][:, 0:512],
                              in_=xqT[kc * 128:(kc + 1) * 128, 0:512])
        for kc in range(KC):
            nc.sync.dma_start(out=xqs[kc][:, 512:T],
                              in_=xqT[kc * 128:(kc + 1) * 128, 512:T])
        wts = [w_pool.tile([128, D_MODEL], BF16, tag=f"wt{c}", name=f"wt{c}")
               for c in range(CAT // 128)]
        for c in range(CAT // 128):
            nc.sync.dma_start(out=wts[c], in_=wtailT[c * 128:(c + 1) * 128, :])

        # ---- persistent intermediates
        qts = [qk_pool.tile([128, T], BF16, tag=f"qt{j}", name=f"qt{j}")
               for j in range(NPAIR)]
        kts = [qk_pool.tile([128, KT], BF16, tag=f"kt{j}", name=f"kt{j}")
               for j in range(NPAIR)]
        vaugs = [va_pool.tile([128, HPC, D_HEAD + 1], BF16, tag=f"va{t}",
                              name=f"va{t}") for t in range(nkb)]
        nums = [num_pool.tile([128, T], BF16, tag=f"nm{j}", name=f"nm{j}")
                for j in range(NPAIR)]

        # ---- work quanta (each: one aux-psum accumulation group + evac)
        def vproj(tb):
            vp = aux_pool.tile([128, 512], F32, tag="aux", bufs=2)
            for kc in range(KC):
                nc.tensor.matmul(vp, xks[kc][:, tb * 128:(tb + 1) * 128],
                                 wvs[kc], start=(kc == 0), stop=(kc == KC - 1))
            va = vaugs[tb]
            nc.vector.tensor_scalar_mul(
                va[:, :, 0:D_HEAD],
                vp.rearrange("p (h d) -> p h d", h=HPC),
                keepc[:, tb:tb + 1])
            nc.vector.tensor_scalar_mul(
                va[:, :, D_HEAD:D_HEAD + 1].rearrange("p h o -> p (h o)"),
                ones8t,
                keepc[:, tb:tb + 1])

        def kproj(j, c):
            n0 = c * 512
            n1 = min(n0 + 512, KT)
            kp = aux_pool.tile([128, 512], F32, tag="aux", bufs=2)
            for kc in range(KC):
                nc.tensor.matmul(kp[:, 0:n1 - n0], wks[j][:, kc, :],
                                 xks[kc][:, n0:n1],
                                 start=(kc == 0), stop=(kc == KC - 1))
            nc.vector.tensor_copy(out=kts[j][:, n0:n1], in_=kp[:, 0:n1 - n0])

        def qproj(j, n):
            n0 = n * 512
            qp = aux_pool.tile([128, 512], F32, tag="aux", bufs=2)
            for kc in range(KC):
                nc.tensor.matmul(qp, wqs[j][:, kc, :], xqs[kc][:, n0:n0 + 512],
                                 start=(kc == 0), stop=(kc == KC - 1))
            nc.vector.tensor_copy(out=qts[j][:, n0:n0 + 512], in_=qp)

        def tailq(tb, n):
            n0 = n * 512
            yp = aux_pool.tile([128, 512], F32, tag="aux", bufs=2)
            for c in range(NPAIR):
                nc.tensor.matmul(yp, nums[c][:, tb * 128:(tb + 1) * 128],
                                 wts[c][:, n0:n0 + 512],
                                 start=(c == 0), stop=(c == NPAIR - 1))
            y_sb = ysb_pool.tile([128, 512], F32, tag="ys", bufs=2)
            nc.vector.tensor_copy(out=y_sb, in_=yp)
            nc.sync.dma_start(out=y[tb * 128:(tb + 1) * 128, n0:n0 + 512],
                              in_=y_sb)

        # general work deque (no intra-unit deadline; consumed 1 per slot)
        deque = []
        for n in range(1, NQB):
            for j in range(NPAIR):
                deque.append((lambda jj=j, nn=n: qproj(jj, nn)))

        def pop_deque():
            if deque:
                deque.pop(0)()

        # ---- attention unit
        def unit(u, j, qb):
            q0 = qb * 512
            h0, h1 = 2 * j, 2 * j + 1
            avpA = avp_pool.tile([D_HEAD + 1, 512], F32, tag="avpA",
                                 name="avpA")
            avpB = avp_pool.tile([D_HEAD + 1, 512], F32, tag="avpB",
                                 name="avpB")
            ps = {}

            def st_exp(kb):
                stp = stp_pool.tile([128, 2, 512], F32, tag="stp", bufs=2)
                nc.tensor.matmul(
                    stp[:, 0, :], kts[j][0:64, kb * 128:(kb + 1) * 128],
                    qts[j][0:64, q0:q0 + 512], start=True, stop=True,
                    tile_position=(0, 0))
                nc.tensor.matmul(
                    stp[:, 1, :], kts[j][64:128, kb * 128:(kb + 1) * 128],
                    qts[j][64:128, q0:q0 + 512], start=True, stop=True,
                    tile_position=(64, 0))
                p = p_pool.tile([128, 2, 512], BF16, tag="p", bufs=3)
                nc.scalar.activation(
                    out=p, in_=stp, func=mybir.ActivationFunctionType.Exp,
                    scale=0.125)
                ps[kb] = p

            def av(kb):
                p = ps.pop(kb)
                nc.tensor.matmul(avpA, vaugs[kb][:, h0, :], p[:, 0, :],
                                 start=(kb == 0), stop=(kb == nkb - 1))
                nc.tensor.matmul(avpB, vaugs[kb][:, h1, :], p[:, 1, :],
                                 start=(kb == 0), stop=(kb == nkb - 1))

            st_exp(0)
            for kb in range(1, nkb):
                # scheduled filler work for this slot
                if u == 0:
                    if kb == 1:
                        for c in range(1, NKC):
                            kproj(0, c)
                    if 2 <= kb + 1 < nkb:
                        vproj(kb + 1)
                elif u in (1, 2, 3):
                    if 1 <= kb < NKC:
                        kproj(j, kb)
                    elif kb % 2 == 1:
                        pop_deque()
                else:
                    if kb % 2 == 1:
                        pop_deque()
                st_exp(kb)
                av(kb - 1)
            av(nkb - 1)

            # normalize + evacuate into nums[j]
            stagA = stag_pool.tile([D_HEAD + 1, 512], F32, tag="sgA", bufs=2)
            stagB = stag_pool.tile([D_HEAD + 1, 512], F32, tag="sgB", bufs=2)
            nc.vector.tensor_copy(out=stagA, in_=avpA)
            nc.vector.tensor_copy(out=stagB, in_=avpB)
            rab = rab_pool.tile([33, 512], F32R, tag="rab", bufs=2)
            with nc.allow_low_precision(reason="f32r holds f32 bits"):
                nc.vector.reciprocal(out=rab[0:1, :],
                                     in_=stagA[D_HEAD:D_HEAD + 1, :])
                nc.vector.reciprocal(out=rab[32:33, :],
                                     in_=stagB[D_HEAD:D_HEAD + 1, :])
            rbp = aux_pool.tile([128, 512], F32, tag="aux", bufs=2)
            nc.tensor.matmul(rbp, selt, rab, start=True, stop=True)
            nc.vector.tensor_tensor(
                out=nums[j][0:64, q0:q0 + 512], in0=stagA[0:64, :],
                in1=rbp[0:64, :], op=mybir.AluOpType.mult)
            nc.vector.tensor_tensor(
                out=nums[j][64:128, q0:q0 + 512], in0=stagB[0:64, :],
                in1=rbp[64:128, :], op=mybir.AluOpType.mult)

        # ---- prologue: minimum to start unit(p0, qb0)
        vproj(0)
        if nkb > 1:
            vproj(1)
        kproj(0, 0)
        qproj(0, 0)

        u = 0
        for qb in range(NQB):
            for j in range(NPAIR):
                if u in (1, 2, 3):
                    kproj(j, 0)
                    qproj(j, 0)
                unit(u, j, qb)
                u += 1
            for tb in range(qb * 4, (qb + 1) * 4):
                for n in range(2):
                    deque.append((lambda t=tb, nn=n: tailq(t, nn)))
        while deque:
            pop_deque()

    if split_waits:
        split_excess_waits(nc)
    return nc


_NC_CACHE = {}


def _get_nc(nkb):
    if nkb not in _NC_CACHE:
        _NC_CACHE[nkb] = build_nc(nkb)
    return _NC_CACHE[nkb]


def make_in_maps(x, mask, w_qkv, w_tail):
    """Shard full inputs into 8 per-core input maps (with key gather)."""
    x = np.asarray(x, dtype=np.float32)
    mask = np.asarray(mask, dtype=np.int32)
    w_qkv = np.asarray(w_qkv, dtype=np.float32)
    w_tail = np.asarray(w_tail, dtype=np.float32)

    # per-batch kept-key gather
    idxs = [np.nonzero(mask[b] != 0)[0] for b in range(BN)]
    nkb = max(4, max((len(ix) + 127) // 128 for ix in idxs))
    KT = nkb * 128

    xk_all, keep_all = [], []
    for b in range(BN):
        ix = idxs[b]
        m = len(ix)
        xk = np.zeros((KT, D_MODEL), dtype=np.float32)
        xk[:m] = x[b][ix]
        kp = np.zeros((KT,), dtype=np.float32)
        kp[:m] = 1.0
        xk_all.append(xk)
        keep_all.append(kp)

    w3 = w_qkv.reshape(N_HEAD, 3, D_HEAD, D_MODEL)  # [head, q|k|v, d, dm]
    selv = np.zeros((33, 128), np.float32)
    selv[0, 0:64] = 1.0
    selv[32, 64:128] = 1.0

    in_maps = []
    for c in range(8):
        b, hg = c // 2, c % 2
        heads = [hg * HPC + i for i in range(HPC)]
        wq = np.concatenate([w3[h, 0] for h in heads], axis=0)  # [512, 1024]
        wk = np.concatenate([w3[h, 1] for h in heads], axis=0)
        wv = np.concatenate([w3[h, 2] for h in heads], axis=0)
        wt = w_tail[:, hg * CAT:(hg + 1) * CAT]  # [1024, 512]
        in_maps.append({
            "xqT": np.ascontiguousarray(x[b].T).astype(BF),
            "xkT": np.ascontiguousarray(xk_all[b].T).astype(BF),
            "wqT": np.ascontiguousarray(wq.T).astype(BF),
            "wkT": np.ascontiguousarray(wk.T).astype(BF),
            "wvT": np.ascontiguousarray(wv.T).astype(BF),
            "wtailT": np.ascontiguousarray(wt.T).astype(BF),
            "keep": keep_all[b],
            "ones8": np.ones((128, HPC), dtype=BF),
            "sel": selv,
        })
    return in_maps, nkb


def kernel(x, mask, w_qkv, w_tail, b_tail):
    in_maps, nkb = make_in_maps(x, mask, w_qkv, w_tail)
    nc = _get_nc(nkb)
    last_err = None
    for _attempt in range(3):
        try:
            res = run_bass_kernel_spmd(nc, in_maps, list(range(8))).results
            break
        except Exception as e:  # transient device/runtime errors: retry
            last_err = e
            _time.sleep(3.0)
    else:
        raise last_err
    out = np.empty((BN, T, D_MODEL), dtype=np.float32)
    b_tail = np.asarray(b_tail, dtype=np.float32)
    for b in range(BN):
        out[b] = res[2 * b]["y"] + res[2 * b + 1]["y"] + b_tail
    return out


# revision 11
# speedup vs baseline: 2.1648x; 1.5039x over previous
"""Multi-head attention Trainium2 kernel, 8-way sharded. v3.

Problem: x[4,2048,1024] -> qkv proj (w_qkv [3072,1024]) -> 16-head attention
with key-padding mask -> tail proj (w_tail [1024,1024]) + b_tail.

Sharding: 8 shards = 4 batches x 2 head-groups (8 heads each = 4 head PAIRS).
Host unshards: out[b] = y_part[2b] + y_part[2b+1] + b_tail.  No collectives.

Key ideas (vs 603 us v1 baseline):
- Host-side key gather: masked keys (~50%) are dropped before the kernel;
  only ceil(max_kept/128) key blocks are computed. Mask correctness comes
  from zeroing V rows and the denominator-ones column for pad keys, so exp
  needs no bias at all.
- bf16 matmul operands everywhere (fp32r matmuls self-load weights serially,
  ~175 ns each; bf16 gets separate LDWEIGHTS that the PE reorder window
  hides under the previous matmul's stream).
- ST row-tiling: the two heads of a pair have K=64 contractions, placed in
  array rows 0-63 / 64-127 via tile_position -> both score matmuls run
  concurrently.
- One exp per (pair, kb, qb): N=1024 covering both heads' [128 keys x 512 q]
  scores in a 2-bank PSUM tile.
- Normalization fully off the critical path: attn^T and the denominator row
  are staged to SBUF per unit; denominator rows of a whole qb round are
  gathered (SBUF->SBUF DMA) into one [8,512] tile, reciprocal'd ONCE
  (DVE reciprocal is ~6.5 ns/elem - batching is essential), broadcast with
  a tiny selector matmul, and applied during the next round's slack.
- Single software-pipelined emission stream with explicit deadlines:
  projection / normalize / tail quanta are injected between attention steps
  so the PE fills the slack under the Act(exp) stream; phases overlap.
- PE warm-up matmuls during the DMA prologue keep the HAM clock gate at
  full rate from the first real matmul on.
"""

import time as _time

import numpy as np
import ml_dtypes
from contextlib import ExitStack

import concourse.bass as bass
import concourse.mybir as mybir
import concourse.tile as tile
from concourse.bass_utils import run_bass_kernel_spmd

# ---------------------------------------------------------------------------
# walrus in this env accepts at most 2 sync waits per instruction; Tile's
# scheduler emits up to 10. Post-pass: peel excess waits onto same-engine
# NoOps inserted immediately before the offending instruction (same engine
# stream position => identical synchronization semantics).
MAX_WAITS = 1


def split_excess_waits(nc):
    for fn in nc.m.functions:
        for bb in fn.blocks:
            insts = list(bb.instructions)
            out = []
            changed = False
            for inst in insts:
                si = inst.sync_info
                waits = list(si.on_wait) if si is not None else []
                if len(waits) > MAX_WAITS:
                    extra = waits[:-MAX_WAITS]
                    for ci in range(0, len(extra), MAX_WAITS):
                        chunk = extra[ci:ci + MAX_WAITS]
                        nop = mybir.InstNoOp(
                            name=f"{inst.name}-ws{ci}", ins=[], outs=[])
                        nop.engine = inst.engine
                        nop.sync_info = mybir.SyncInfo(
                            on_wait=chunk, on_update=[])
                        out.append(nop)
                    inst.sync_info = mybir.SyncInfo(
                        on_wait=waits[-MAX_WAITS:],
                        on_update=list(si.on_update))
                    changed = True
                out.append(inst)
            if changed:
                bb.instructions = out
# ---------------------------------------------------------------------------

D_MODEL = 1024
N_HEAD = 16
D_HEAD = 64
BN, T = 4, 2048
HPC = 8                      # heads per core
NPAIR = HPC // 2             # 4 head pairs per core
CAT = HPC * D_HEAD           # 512 per-core tail contraction
KC = D_MODEL // 128          # 8 contraction chunks
NQB = T // 512               # 4 q blocks of 512
F32 = mybir.dt.float32
F32R = mybir.dt.float32r
BF16 = mybir.dt.bfloat16
BF = ml_dtypes.bfloat16


def build_nc(nkb, split_waits=True):
    """nkb: number of 128-key blocks of gathered (kept+pad) keys."""
    KT = nkb * 128
    NKC = (KT + 511) // 512      # kproj 512-col chunks
    nc = bass.Bass()
    xqT = nc.declare_dram_parameter("xqT", [D_MODEL, T], BF16, isOutput=False)
    xkT = nc.declare_dram_parameter("xkT", [D_MODEL, KT], BF16, isOutput=False)
    wqT = nc.declare_dram_parameter("wqT", [D_MODEL, NPAIR * 128], BF16, isOutput=False)
    wkT = nc.declare_dram_parameter("wkT", [D_MODEL, NPAIR * 128], BF16, isOutput=False)
    wvT = nc.declare_dram_parameter("wvT", [D_MODEL, CAT], BF16, isOutput=False)
    wtailT = nc.declare_dram_parameter("wtailT", [CAT, D_MODEL], BF16, isOutput=False)
    keep = nc.declare_dram_parameter("keep", [KT], F32, isOutput=False)
    ones8 = nc.declare_dram_parameter("ones8", [128, HPC], BF16, isOutput=False)
    sel4 = nc.declare_dram_parameter("sel4", [HPC, NPAIR * 128], F32R, isOutput=False)
    warm = nc.declare_dram_parameter("warm", [128, 128], BF16, isOutput=False)
    y = nc.declare_dram_parameter("y", [T, D_MODEL], F32, isOutput=True)

    with ExitStack() as ctx:
        tc = ctx.enter_context(tile.TileContext(nc))

        const = ctx.enter_context(tc.tile_pool(name="const", bufs=1))
        xq_pool = ctx.enter_context(tc.tile_pool(name="xq", bufs=1))
        xk_pool = ctx.enter_context(tc.tile_pool(name="xk", bufs=1))
        qk_pool = ctx.enter_context(tc.tile_pool(name="qk", bufs=1))
        va_pool = ctx.enter_context(tc.tile_pool(name="va", bufs=1))
        num_pool = ctx.enter_context(tc.tile_pool(name="num", bufs=1))
        w_pool = ctx.enter_context(tc.tile_pool(name="w", bufs=1))
        p_pool = ctx.enter_context(tc.tile_pool(name="p", bufs=1))
        stag_pool = ctx.enter_context(tc.tile_pool(name="stag", bufs=1))
        rb_pool = ctx.enter_context(tc.tile_pool(name="rb", bufs=1))
        ysb_pool = ctx.enter_context(tc.tile_pool(name="ysb", bufs=1))
        stp_pool = ctx.enter_context(tc.tile_pool(name="stp", bufs=1, space="PSUM"))
        avp_pool = ctx.enter_context(tc.tile_pool(name="avp", bufs=1, space="PSUM"))
        aux_pool = ctx.enter_context(tc.tile_pool(name="aux", bufs=1, space="PSUM"))

        # ---- consts (warm tile first: PE warm-up starts as soon as it lands)
        warmt = const.tile([128, 128], BF16, name="warm")
        nc.sync.dma_start(out=warmt, in_=warm[:, :])
        sel4t = const.tile([HPC, NPAIR, 128], F32R, name="sel4")
        nc.sync.dma_start(
            out=sel4t, in_=sel4.rearrange("p (j c) -> p j c", c=128))
        keepc = const.tile([128, nkb], F32, name="keep")
        nc.sync.dma_start(out=keepc, in_=keep.rearrange("(j p) -> p j", p=128))
        ones8t = const.tile([128, HPC], BF16, name="ones8")
        nc.sync.dma_start(out=ones8t, in_=ones8[:, :])

        # ---- PE warm-up: ~4us of dummy matmuls so HAM reaches K=8/8 before
        # the first real work; overlaps the input DMA wave.
        for g in range(4):
            wp = aux_pool.tile([128, 512], F32, tag="aux", bufs=2)
            for i in range(12):
                nc.tensor.matmul(wp[:, 0:128], warmt, warmt,
                                 start=(i == 0), stop=(i == 11))

        # ---- inputs (order = DMA priority: kproj/qproj first wave)
        wks = [w_pool.tile([128, KC, 128], BF16, tag=f"wk{j}", name=f"wk{j}")
               for j in range(NPAIR)]
        wqs = [w_pool.tile([128, KC, 128], BF16, tag=f"wq{j}", name=f"wq{j}")
               for j in range(NPAIR)]
        xks = [xk_pool.tile([128, KT], BF16, tag=f"xk{kc}", name=f"xk{kc}")
               for kc in range(KC)]
        xqs = [xq_pool.tile([128, T], BF16, tag=f"xq{kc}", name=f"xq{kc}")
               for kc in range(KC)]
        nc.sync.dma_start(
            out=wks[0],
            in_=wkT.rearrange("(kc p) c -> p kc c", p=128)[:, :, 0:128])
        for kc in range(KC):
            nc.sync.dma_start(out=xks[kc][:, 0:512],
                              in_=xkT[kc * 128:(kc + 1) * 128, 0:512])
        nc.sync.dma_start(
            out=wqs[0],
            in_=wqT.rearrange("(kc p) c -> p kc c", p=128)[:, :, 0:128])
        for kc in range(KC):
            nc.sync.dma_start(out=xqs[kc][:, 0:512],
                              in_=xqT[kc * 128:(kc + 1) * 128, 0:512])
        wvs = [w_pool.tile([128, CAT], BF16, tag=f"wv{kc}", name=f"wv{kc}")
               for kc in range(KC)]
        for kc in range(KC):
            nc.sync.dma_start(out=wvs[kc], in_=wvT[kc * 128:(kc + 1) * 128, :])
        if KT > 512:
            for kc in range(KC):
                nc.sync.dma_start(out=xks[kc][:, 512:KT],
                                  in_=xkT[kc * 128:(kc + 1) * 128, 512:KT])
        for j in range(1, NPAIR):
            nc.sync.dma_start(
                out=wks[j],
                in_=wkT.rearrange("(kc p) c -> p kc c", p=128)[
                    :, :, j * 128:(j + 1) * 128])
            nc.sync.dma_start(
                out=wqs[j],
                in_=wqT.rearrange("(kc p) c -> p kc c", p=128)[
                    :, :, j * 128:(j + 1) * 128])
        for kc in range(KC):
            nc.sync.dma_start(out=xqs[kc][:, 512:T],
                              in_=xqT[kc * 128:(kc + 1) * 128, 512:T])
        wts = [w_pool.tile([128, D_MODEL], BF16, tag=f"wt{c}", name=f"wt{c}")
               for c in range(CAT // 128)]
        for c in range(CAT // 128):
            nc.sync.dma_start(out=wts[c], in_=wtailT[c * 128:(c + 1) * 128, :])

        # ---- persistent intermediates
        qts = [qk_pool.tile([128, T], BF16, tag=f"qt{j}", name=f"qt{j}")
               for j in range(NPAIR)]
        kts = [qk_pool.tile([128, KT], BF16, tag=f"kt{j}", name=f"kt{j}")
               for j in range(NPAIR)]
        vaugs = [va_pool.tile([128, HPC, D_HEAD + 1], BF16, tag=f"va{t}",
                              name=f"va{t}") for t in range(nkb)]
        nums = [num_pool.tile([128, T], BF16, tag=f"nm{j}", name=f"nm{j}")
                for j in range(NPAIR)]
        # per-pair staged attn^T (+denominator row); written each round,
        # read by the deferred normalize during the NEXT round => bufs=2.
        def stag(j, h):
            return stag_pool.tile([D_HEAD + 1, 512], F32, tag=f"sg{j}{h}",
                                  name=f"sg{j}{h}", bufs=2)

        # ---- work quanta (each: one aux-psum accumulation group + evac)
        def vproj(tb):
            vp = aux_pool.tile([128, 512], F32, tag="aux", bufs=2)
            for kc in range(KC):
                nc.tensor.matmul(vp, xks[kc][:, tb * 128:(tb + 1) * 128],
                                 wvs[kc], start=(kc == 0), stop=(kc == KC - 1))
            va = vaugs[tb]
            nc.vector.tensor_scalar_mul(
                va[:, :, 0:D_HEAD],
                vp.rearrange("p (h d) -> p h d", h=HPC),
                keepc[:, tb:tb + 1])
            nc.vector.tensor_scalar_mul(
                va[:, :, D_HEAD:D_HEAD + 1].rearrange("p h o -> p (h o)"),
                ones8t,
                keepc[:, tb:tb + 1])

        def kproj(j, c):
            n0 = c * 512
            n1 = min(n0 + 512, KT)
            kp = aux_pool.tile([128, 512], F32, tag="aux", bufs=2)
            for kc in range(KC):
                nc.tensor.matmul(kp[:, 0:n1 - n0], wks[j][:, kc, :],
                                 xks[kc][:, n0:n1],
                                 start=(kc == 0), stop=(kc == KC - 1))
            nc.vector.tensor_copy(out=kts[j][:, n0:n1], in_=kp[:, 0:n1 - n0])

        def qproj(j, n):
            n0 = n * 512
            qp = aux_pool.tile([128, 512], F32, tag="aux", bufs=2)
            for kc in range(KC):
                nc.tensor.matmul(qp, wqs[j][:, kc, :], xqs[kc][:, n0:n0 + 512],
                                 start=(kc == 0), stop=(kc == KC - 1))
            nc.vector.tensor_copy(out=qts[j][:, n0:n0 + 512], in_=qp)

        def tailq(tb, n):
            n0 = n * 512
            yp = aux_pool.tile([128, 512], F32, tag="aux", bufs=2)
            for c in range(NPAIR):
                nc.tensor.matmul(yp, nums[c][:, tb * 128:(tb + 1) * 128],
                                 wts[c][:, n0:n0 + 512],
                                 start=(c == 0), stop=(c == NPAIR - 1))
            y_sb = ysb_pool.tile([128, 512], F32, tag="ys", bufs=2)
            nc.vector.tensor_copy(out=y_sb, in_=yp)
            nc.sync.dma_start(out=y[tb * 128:(tb + 1) * 128, n0:n0 + 512],
                              in_=y_sb)

        # ---- deferred normalization for one qb round. The reciprocal (DVE,
        # ~6.5 ns/elem, batched to one [8,512] op/round) is emitted at round
        # end so it runs during the next round's attention; the broadcast +
        # apply quanta are consumed from the deque well after it finished.
        def emit_recip(rball):
            rtmp = rb_pool.tile([HPC, 512], F32, tag="rtmp", bufs=2)
            nc.vector.reciprocal(out=rtmp, in_=rball)
            rrec = rb_pool.tile([HPC, 512], F32R, tag="rrec", bufs=2)
            nc.vector.tensor_copy(out=rrec, in_=rtmp)
            return rrec

        def make_norm_quanta(qb, rrec, round_stags):
            q0 = qb * 512

            def normj(j):
                rbp = aux_pool.tile([128, 512], F32, tag="aux", bufs=2)
                nc.tensor.matmul(rbp, sel4t[:, j, :], rrec,
                                 start=True, stop=True)
                nc.vector.tensor_tensor(
                    out=nums[j][0:64, q0:q0 + 512],
                    in0=round_stags[j][0][0:64, :],
                    in1=rbp[0:64, :], op=mybir.AluOpType.mult)
                nc.vector.tensor_tensor(
                    out=nums[j][64:128, q0:q0 + 512],
                    in0=round_stags[j][1][0:64, :],
                    in1=rbp[64:128, :], op=mybir.AluOpType.mult)

            return [lambda jj=j: normj(jj) for j in range(NPAIR)]

        # general work deque (FIFO; consumed one per slot)
        deque = []
        for n in range(1, NQB):
            for j in range(NPAIR):
                deque.append((lambda jj=j, nn=n: qproj(jj, nn)))

        def pop_deque():
            if deque:
                deque.pop(0)()

        # ---- attention unit
        def unit(u, j, qb, rball):
            q0 = qb * 512
            h0, h1 = 2 * j, 2 * j + 1
            avpA = avp_pool.tile([D_HEAD + 1, 512], F32, tag="avpA",
                                 name="avpA")
            avpB = avp_pool.tile([D_HEAD + 1, 512], F32, tag="avpB",
                                 name="avpB")
            ps = {}

            def st_exp(kb):
                stp = stp_pool.tile([128, 2, 512], F32, tag="stp", bufs=2)
                nc.tensor.matmul(
                    stp[:, 0, :], kts[j][0:64, kb * 128:(kb + 1) * 128],
                    qts[j][0:64, q0:q0 + 512], start=True, stop=True,
                    tile_position=(0, 0))
                nc.tensor.matmul(
                    stp[:, 1, :], kts[j][64:128, kb * 128:(kb + 1) * 128],
                    qts[j][64:128, q0:q0 + 512], start=True, stop=True,
                    tile_position=(64, 0))
                p = p_pool.tile([128, 2, 512], BF16, tag="p", bufs=3)
                nc.scalar.activation(
                    out=p, in_=stp, func=mybir.ActivationFunctionType.Exp,
                    scale=0.125)
                ps[kb] = p

            def av(kb):
                p = ps.pop(kb)
                nc.tensor.matmul(avpA, vaugs[kb][:, h0, :], p[:, 0, :],
                                 start=(kb == 0), stop=(kb == nkb - 1))
                nc.tensor.matmul(avpB, vaugs[kb][:, h1, :], p[:, 1, :],
                                 start=(kb == 0), stop=(kb == nkb - 1))

            st_exp(0)
            for kb in range(1, nkb):
                # scheduled filler work for this slot
                if u == 0:
                    if kb == 1:
                        vproj(0)
                        vproj(1)
                        vproj(2)
                    elif kb <= NKC:
                        kproj(0, kb - 1)
                        if kb + 1 < nkb:
                            vproj(kb + 1)
                    elif kb + 1 < nkb:
                        vproj(kb + 1)
                elif u in (1, 2, 3):
                    if 1 <= kb < NKC:
                        kproj(j, kb)
                    else:
                        pop_deque()
                else:
                    pop_deque()
                st_exp(kb)
                av(kb - 1)
            av(nkb - 1)

            # stage attn^T + denominator row; gather D rows for the round
            sgA, sgB = stag(j, 0), stag(j, 1)
            nc.vector.tensor_copy(out=sgA, in_=avpA)
            nc.vector.tensor_copy(out=sgB, in_=avpB)
            nc.sync.dma_start(out=rball[h0:h0 + 1, :],
                              in_=sgA[D_HEAD:D_HEAD + 1, :])
            nc.sync.dma_start(out=rball[h1:h1 + 1, :],
                              in_=sgB[D_HEAD:D_HEAD + 1, :])
            return sgA, sgB

        # ---- prologue: minimum to start unit(p0, qb0)
        kproj(0, 0)
        qproj(0, 0)

        u = 0
        for qb in range(NQB):
            rball = rb_pool.tile([HPC, 512], F32, tag="rball", bufs=2)
            round_stags = []
            for j in range(NPAIR):
                if u in (1, 2, 3):
                    kproj(j, 0)
                    qproj(j, 0)
                round_stags.append(unit(u, j, qb, rball))
                u += 1
            rrec = emit_recip(rball)
            # tail(qb-1) first (its norms are done), then norm(qb): keeps
            # the rb matmuls far behind the reciprocal they wait on.
            if qb >= 1:
                for tb in range((qb - 1) * 4, qb * 4):
                    for n in range(2):
                        deque.append((lambda t=tb, nn=n: tailq(t, nn)))
            deque.extend(make_norm_quanta(qb, rrec, round_stags))
        for tb in range((NQB - 1) * 4, NQB * 4):
            for n in range(2):
                deque.append((lambda t=tb, nn=n: tailq(t, nn)))
        while deque:
            pop_deque()

    if split_waits:
        split_excess_waits(nc)
    return nc


_NC_CACHE = {}


def _get_nc(nkb):
    if nkb not in _NC_CACHE:
        _NC_CACHE[nkb] = build_nc(nkb)
    return _NC_CACHE[nkb]


def make_in_maps(x, mask, w_qkv, w_tail):
    """Shard full inputs into 8 per-core input maps (with key gather)."""
    x = np.asarray(x, dtype=np.float32)
    mask = np.asarray(mask, dtype=np.int32)
    w_qkv = np.asarray(w_qkv, dtype=np.float32)
    w_tail = np.asarray(w_tail, dtype=np.float32)

    # per-batch kept-key gather
    idxs = [np.nonzero(mask[b] != 0)[0] for b in range(BN)]
    nkb = max(4, max((len(ix) + 127) // 128 for ix in idxs))
    KT = nkb * 128

    xk_all, keep_all = [], []
    for b in range(BN):
        ix = idxs[b]
        m = len(ix)
        xk = np.zeros((KT, D_MODEL), dtype=np.float32)
        xk[:m] = x[b][ix]
        kp = np.zeros((KT,), dtype=np.float32)
        kp[:m] = 1.0
        xk_all.append(xk)
        keep_all.append(kp)

    w3 = w_qkv.reshape(N_HEAD, 3, D_HEAD, D_MODEL)  # [head, q|k|v, d, dm]
    # sel4[:, j*128:(j+1)*128]: broadcast selector for pair j
    selv = np.zeros((HPC, NPAIR * 128), np.float32)
    for j in range(NPAIR):
        selv[2 * j, j * 128:j * 128 + 64] = 1.0
        selv[2 * j + 1, j * 128 + 64:(j + 1) * 128] = 1.0

    in_maps = []
    for c in range(8):
        b, hg = c // 2, c % 2
        heads = [hg * HPC + i for i in range(HPC)]
        wq = np.concatenate([w3[h, 0] for h in heads], axis=0)  # [512, 1024]
        wk = np.concatenate([w3[h, 1] for h in heads], axis=0)
        wv = np.concatenate([w3[h, 2] for h in heads], axis=0)
        wt = w_tail[:, hg * CAT:(hg + 1) * CAT]  # [1024, 512]
        in_maps.append({
            "xqT": np.ascontiguousarray(x[b].T).astype(BF),
            "xkT": np.ascontiguousarray(xk_all[b].T).astype(BF),
            "wqT": np.ascontiguousarray(wq.T).astype(BF),
            "wkT": np.ascontiguousarray(wk.T).astype(BF),
            "wvT": np.ascontiguousarray(wv.T).astype(BF),
            "wtailT": np.ascontiguousarray(wt.T).astype(BF),
            "keep": keep_all[b],
            "ones8": np.ones((128, HPC), dtype=BF),
            "sel4": selv,
            "warm": np.zeros((128, 128), dtype=BF),
        })
    return in_maps, nkb


def kernel(x, mask, w_qkv, w_tail, b_tail):
    in_maps, nkb = make_in_maps(x, mask, w_qkv, w_tail)
    nc = _get_nc(nkb)
    last_err = None
    for _attempt in range(3):
        try:
            res = run_bass_kernel_spmd(nc, in_maps, list(range(8))).results
            break
        except Exception as e:  # transient device/runtime errors: retry
            last_err = e
            _time.sleep(3.0)
    else:
        raise last_err
    out = np.empty((BN, T, D_MODEL), dtype=np.float32)
    b_tail = np.asarray(b_tail, dtype=np.float32)
    for b in range(BN):
        out[b] = res[2 * b]["y"] + res[2 * b + 1]["y"] + b_tail
    return out
